# revision 8
# baseline (speedup 1.0000x reference)
"""Trainium2 Bass kernel for nn_Attention_15556371546220 (Enformer-style
relative-position attention, B=1 L=4096 C=768 H=4 DK=64 DV=192 POSF=64).

Sharding: 8 cores = 4 heads x 2 query-blocks of 2048. Each core computes its
head's K/V over the full sequence, Q over its query block, full attention with
the relative-shift positional term, and a partial output projection
(row-parallel over the head's 192 value dims). Host gathers: sums the 4 head
partials per query block and adds the output bias.

v2 design (transposed-attention pipeline):
- Relative shift: per query tile qt, U[p, m] = y_p . pk[ws+m] (width 4223) is
  written to DRAM with a SKEWED access pattern (partition stride USR+1) so the
  DRAM buffer holds, at addr i*USR + 128 + j, the shifted value
  y_i . pk[j - i + 2047].  The readback is then a PLAIN 2D pattern
  [[USR, nq], [1, 128]] fed through the DMA XBAR TRANSPOSE, producing pos
  logits already transposed: posT[k, q].
- Content logits are computed transposed directly (lhsT = k tile [64, 128],
  rhs = qct chunk [64, 512]); K=64 row groups 0-63, while the U matmuls use
  row groups 64-127 (pkt/qpt live on partitions 64-127) so the PE can run
  both concurrently (row tiling).
- attnT = exp(contentT + posT) comes out already in [key, query] layout, so
  the O = V^T @ attnT accumulation needs NO PE transposes (the baseline spent
  ~150us on 512 of them).
- Softmax denominators come free from a ones-column appended to V (row 64 of
  o2 PSUM); they are transposed to per-query-partition scale factors with 4
  tiny K=1 PE transposes per query chunk and applied in the output projection.
"""
import sys
if "/opt/trn_rl_repo" not in sys.path:
    sys.path.insert(0, "/opt/trn_rl_repo")

import os
import numpy as np
import ml_dtypes

import concourse.bass as bass
import concourse.bacc as bacc
import concourse.mybir as mybir
import concourse.tile as tile
from concourse.bass_utils import run_bass_kernel_spmd

F32 = mybir.dt.float32
BF16 = mybir.dt.bfloat16
FP16 = mybir.dt.float16
AX = mybir.AxisListType
ALU = mybir.AluOpType
ACT = mybir.ActivationFunctionType

B, L, C = 1, 4096, 768
H, DK, DV = 4, 64, 192
POSF = 64
NQ = 2048          # queries per core (one of two blocks)
UW = 4223          # U tile width
USR = 4352         # U DRAM row pitch (elements); >= UW + 128 to avoid spill
PKW = 6144         # per-core pos-key window (covers all 16 tiles)

XR = int(os.environ.get("KXR", "512"))   # xbar transpose rows per instr

_nc_cache = {}


def _build_nc():
    nc = bacc.Bacc()

    xt_in = nc.declare_dram_parameter("xt", (C, L), FP16, isOutput=False)
    xq_in = nc.declare_dram_parameter("xq", (C, NQ), FP16, isOutput=False)
    wq_in = nc.declare_dram_parameter("wq", (C, DK), FP16, isOutput=False)
    wk_in = nc.declare_dram_parameter("wk", (C, DK), FP16, isOutput=False)
    wv_in = nc.declare_dram_parameter("wv", (C, DV), FP16, isOutput=False)
    wpos_in = nc.declare_dram_parameter("wpos", (POSF, DK), FP16, isOutput=False)
    post_in = nc.declare_dram_parameter("post", (POSF, PKW), FP16, isOutput=False)
    wout_in = nc.declare_dram_parameter("wout", (DV, C), FP16, isOutput=False)
    rcb_in = nc.declare_dram_parameter("rcb", (DK, 1), F32, isOutput=False)
    rpb_in = nc.declare_dram_parameter("rpb", (DK, 1), F32, isOutput=False)
    out_dram = nc.declare_dram_parameter("out", (NQ, C), BF16, isOutput=True)

    with tile.TileContext(nc) as tc:
        with (
            tc.tile_pool(name="const", bufs=1) as cpool,
            tc.tile_pool(name="res", bufs=1) as rpool,
            tc.tile_pool(name="udram", bufs=2, space="DRAM") as dpool,
        ):
            # ---------- constants ----------
            wq_sb = cpool.tile([128, 6, DK], FP16)
            nc.gpsimd.dma_start(wq_sb[:], wq_in.rearrange("(cc p) d -> p cc d", p=128))
            wk_sb = cpool.tile([128, 6, DK], FP16)
            nc.gpsimd.dma_start(wk_sb[:], wk_in.rearrange("(cc p) d -> p cc d", p=128))
            wv_sb = cpool.tile([128, 6, DV], FP16)
            nc.gpsimd.dma_start(wv_sb[:], wv_in.rearrange("(cc p) d -> p cc d", p=128))
            wpos_sb = cpool.tile([POSF, DK], FP16)
            nc.gpsimd.dma_start(wpos_sb[:], wpos_in[:])
            wout1_sb = cpool.tile([128, C], FP16)
            nc.gpsimd.dma_start(wout1_sb[:], wout_in[0:128, :])
            wout2_sb = cpool.tile([64, C], FP16)
            nc.gpsimd.dma_start(wout2_sb[:], wout_in[128:192, :])
            bias_sb = cpool.tile([128, 1], F32)   # rows 0-63 rcb, 64-127 rpb
            nc.gpsimd.dma_start(bias_sb[0:64, :], rcb_in[:])
            nc.gpsimd.dma_start(bias_sb[64:128, :], rpb_in[:])
            one_sb = cpool.tile([128, 1], F32)
            nc.vector.memset(one_sb[:], 1.0)

            # ---------- residents ----------
            # rows 0-63: K^T (dk x key) in cols 0:4096
            # rows 64-127: pos_k^T (dk x pos window) in cols 0:6144
            kpk_sb = rpool.tile([128, PKW], FP16)
            # rows 0-63: (Q/8 + rcb)^T ; rows 64-127: (Q/8 + rpb)^T
            qq_sb = rpool.tile([128, NQ], FP16)
            v1_sb = rpool.tile([128, 32, 128], FP16)   # V[:, :128] per key block
            v2a_sb = rpool.tile([128, 32, 72], FP16)   # V[:, 128:192] + ones col
            nc.gpsimd.memset(v2a_sb[:, :, 64:65], 1.0)

            # ---------- phase A: projections ----------
            with (
                tc.tile_pool(name="xa", bufs=7) as xpool,
                tc.tile_pool(name="psK", bufs=2, space="PSUM") as psK,
                tc.tile_pool(name="psQ1", bufs=2, space="PSUM") as psQ1,
                tc.tile_pool(name="psQ2", bufs=2, space="PSUM") as psQ2,
                tc.tile_pool(name="psV", bufs=2, space="PSUM") as psV,
            ):
                for kc in range(8):
                    xts = []
                    for cc in range(6):
                        xt_t = xpool.tile([128, 512], FP16, tag="xs")
                        nc.sync.dma_start(
                            xt_t[:], xt_in[cc * 128:(cc + 1) * 128,
                                           kc * 512:(kc + 1) * 512])
                        xts.append(xt_t)
                    kps = psK.tile([64, 512], F32, tag="psk")
                    for cc in range(6):
                        nc.tensor.matmul(kps[:], wk_sb[:, cc, :], xts[cc][:],
                                         start=(cc == 0), stop=(cc == 5))
                    nc.scalar.copy(kpk_sb[0:64, kc * 512:(kc + 1) * 512], kps[:])
                    for sub in range(4):
                        kb = kc * 4 + sub
                        vps = psV.tile([128, DV], F32, tag="psv")
                        for cc in range(6):
                            nc.tensor.matmul(
                                vps[:], xts[cc][:, sub * 128:(sub + 1) * 128],
                                wv_sb[:, cc, :], start=(cc == 0), stop=(cc == 5))
                        if sub % 2 == 0:
                            nc.vector.tensor_copy(v1_sb[:, kb, :],
                                                  vps[:, 0:128])
                            nc.vector.tensor_copy(v2a_sb[:, kb, 0:64],
                                                  vps[:, 128:192])
                        else:
                            nc.scalar.copy(v1_sb[:, kb, :], vps[:, 0:128])
                            nc.scalar.copy(v2a_sb[:, kb, 0:64],
                                           vps[:, 128:192])

                for qc in range(4):
                    q1 = psQ1.tile([64, 512], F32, tag="psq1")
                    q2f = psQ2.tile([128, 512], F32, tag="psq2")
                    xqs = []
                    for cc in range(6):
                        xq_t = xpool.tile([128, 512], FP16, tag="xs")
                        nc.sync.dma_start(
                            xq_t[:], xq_in[cc * 128:(cc + 1) * 128,
                                           qc * 512:(qc + 1) * 512])
                        xqs.append(xq_t)
                    for cc in range(6):
                        nc.tensor.matmul(q1[:], wq_sb[:, cc, :], xqs[cc][:],
                                         start=(cc == 0), stop=(cc == 5))
                    for cc in range(6):
                        nc.tensor.matmul(q2f[64:128, :], wq_sb[:, cc, :],
                                         xqs[cc][:],
                                         start=(cc == 0), stop=(cc == 5))
                    nc.scalar.activation(qq_sb[0:64, qc * 512:(qc + 1) * 512],
                                         q1[:], ACT.Identity,
                                         bias=bias_sb[0:64, :], scale=0.125)
                    nc.scalar.activation(qq_sb[64:128, qc * 512:(qc + 1) * 512],
                                         q2f[64:128, :], ACT.Identity,
                                         bias=bias_sb[64:128, :], scale=0.125)

                for mc in range(12):
                    po_t = xpool.tile([POSF, 512], FP16, tag="po")
                    nc.sync.dma_start(
                        po_t[:], post_in[:, mc * 512:(mc + 1) * 512])
                    pk2 = psQ2.tile([128, 512], F32, tag="psq2")
                    nc.tensor.matmul(pk2[64:128, :], wpos_sb[:], po_t[:],
                                     start=True, stop=True)
                    nc.scalar.copy(kpk_sb[64:128, mc * 512:(mc + 1) * 512],
                                   pk2[64:128, :])

            # ---------- phase B: attention ----------
            with (
                tc.tile_pool(name="ub", bufs=3) as upool,
                tc.tile_pool(name="pos", bufs=6) as pospool,
                tc.tile_pool(name="li", bufs=4) as lipool,
                tc.tile_pool(name="att", bufs=2) as attpool,
                tc.tile_pool(name="ot", bufs=2) as opool,
                tc.tile_pool(name="zz", bufs=2) as zpool,
                tc.tile_pool(name="fin", bufs=3) as fpool,
                tc.tile_pool(name="psU", bufs=2, space="PSUM") as psU,
                tc.tile_pool(name="psC", bufs=2, space="PSUM") as psC,
                tc.tile_pool(name="psO1", bufs=1, space="PSUM") as psO1,
                tc.tile_pool(name="psO2", bufs=1, space="PSUM") as psO2,
                tc.tile_pool(name="psP", bufs=1, space="PSUM") as psP,
                tc.tile_pool(name="psR", bufs=1, space="PSUM") as psR,
            ):
                def emit_u(qt, udr):
                    ws = 1920 - 128 * qt
                    t = qt % 4
                    u_sb = upool.tile([128, UW], FP16, tag="u")
                    for uc in range(9):
                        w = 512 if uc < 8 else UW - 8 * 512
                        ups = psU.tile([128, 512], F32, tag="psu")
                        nc.tensor.matmul(
                            ups[:, 0:w],
                            qq_sb[64:128, qt * 128:(qt + 1) * 128],
                            kpk_sb[64:128, ws + uc * 512: ws + uc * 512 + w],
                            start=True, stop=True)
                        if uc % 2 == 0:
                            nc.vector.tensor_copy(
                                u_sb[:, uc * 512: uc * 512 + w], ups[:, 0:w])
                        else:
                            nc.scalar.copy(
                                u_sb[:, uc * 512: uc * 512 + w], ups[:, 0:w])
                    # skewed write: value for (i = 128qt+p, j) lands at
                    # addr i*USR + 128 + j  (m = j + 127 - p)
                    dst = bass.AP(udr.tensor,
                                  udr.offset + 128 * t * USR + 1,
                                  [[USR + 1, 128], [1, UW]])
                    nc.gpsimd.dma_start(dst, u_sb[:])

                def new_udr():
                    udr_t = dpool.tile([512, USR], FP16, tag="udr",
                                       name="udr_t")
                    return udr_t[:]

                udrs = {0: new_udr()}
                for t in range(4):
                    emit_u(t, udrs[0])

                LAG = 3
                for qch in range(4):
                    udr = udrs[qch]
                    att = attpool.tile([128, 32, 512], BF16, tag="att")
                    o1ps = psO1.tile([128, 512], F32, tag="o1")
                    o2ps = psO2.tile([65, 512], F32, tag="o2")
                    for kb in range(32 + LAG):
                        if kb < 32:
                            pos_t = pospool.tile([128, 512], FP16, tag="pos")
                            for xs in range(512 // XR):
                                src = bass.AP(
                                    udr.tensor,
                                    udr.offset + (xs * XR) * USR
                                    + 128 + 128 * kb,
                                    [[USR, XR], [1, 128]])
                                nc.sync.dma_start(
                                    pos_t[:, xs * XR:(xs + 1) * XR], src,
                                    transpose=True)
                            cps = psC.tile([128, 512], F32, tag="psc")
                            nc.tensor.matmul(
                                cps[:],
                                kpk_sb[0:64, kb * 128:(kb + 1) * 128],
                                qq_sb[0:64, qch * 512:(qch + 1) * 512],
                                start=True, stop=True)
                            li = lipool.tile([128, 512], FP16, tag="li")
                            nc.vector.tensor_add(li[:], cps[:], pos_t[:])
                            nc.scalar.activation(att[:, kb, :], li[:], ACT.Exp)
                        ko = kb - LAG
                        if ko >= 0:
                            nc.tensor.matmul(o1ps[:], v1_sb[:, ko, :],
                                             att[:, ko, :],
                                             start=(ko == 0), stop=(ko == 31))
                            nc.tensor.matmul(o2ps[:], v2a_sb[:, ko, 0:65],
                                             att[:, ko, :],
                                             start=(ko == 0), stop=(ko == 31))
                        if qch < 3 and kb % 8 == 7:
                            if kb == 7:
                                udrs[qch + 1] = new_udr()
                            emit_u(4 * (qch + 1) + kb // 8, udrs[qch + 1])

                    # --- epilogue for this query chunk
                    o1t = opool.tile([128, 512], BF16, tag="o1t")
                    nc.scalar.copy(o1t[:], o1ps[:])
                    o2t = opool.tile([65, 512], BF16, tag="o2t")
                    nc.scalar.copy(o2t[:], o2ps[:])
                    zr = zpool.tile([65, 512], F32, tag="zr")
                    nc.vector.tensor_copy(zr[64:65, :], o2ps[64:65, :])
                    rz4 = zpool.tile([128, 4], F32, tag="rz")
                    for qt2 in range(4):
                        # own rotation slot per transpose: serializes the 4
                        # K=1 PE transposes against their readers so the
                        # start-of-group PSUM clear can't clobber a column
                        # still being read
                        rps = psR.tile([128, 2], F32, tag="rps")
                        nc.tensor.transpose(
                            rps[:, 0:1],
                            zr[64:65, qt2 * 128:(qt2 + 1) * 128],
                            one_sb[64:65, 0:1])
                        nc.vector.reciprocal(rz4[:, qt2:qt2 + 1],
                                             rps[:, 0:1])
                    for qt2 in range(4):
                        fin = fpool.tile([128, C], BF16, tag="fin")
                        for n0 in (0, 384):
                            pps = psP.tile([128, 384], F32, tag="pp")
                            nc.tensor.matmul(
                                pps[:],
                                o1t[:, qt2 * 128:(qt2 + 1) * 128],
                                wout1_sb[:, n0:n0 + 384],
                                start=True, stop=False)
                            nc.tensor.matmul(
                                pps[:],
                                o2t[0:64, qt2 * 128:(qt2 + 1) * 128],
                                wout2_sb[:, n0:n0 + 384],
                                start=False, stop=True)
                            nc.scalar.activation(fin[:, n0:n0 + 384], pps[:],
                                                 ACT.Copy,
                                                 scale=rz4[:, qt2:qt2 + 1])
                        nc.gpsimd.dma_start(
                            out_dram[(qch * 4 + qt2) * 128:
                                     (qch * 4 + qt2 + 1) * 128, :], fin[:])

    nc.finalize()
    return nc


def _positions_T():
    feat = POSF // 2
    pow_rate = np.exp(np.log(L + 1) / feat).astype(np.float64)
    pos = np.arange(-L + 1, L, dtype=np.float64)                 # (8191,)
    cw = pow_rate ** np.arange(1, feat + 1, dtype=np.float64) - 1.0
    emb = (cw[None, :] > np.abs(pos)[:, None]).astype(np.float32)
    signed = np.sign(pos)[:, None].astype(np.float32) * emb
    p = np.concatenate([emb, signed], axis=-1)                   # (8191, 64)
    pt = np.zeros((POSF, 2 * L), np.float32)
    pt[:, :2 * L - 1] = p.T
    return pt


def kernel(x, Wq, Wk, Wv, Wpos, Wout, bout, rel_content_bias, rel_pos_bias):
    f16 = np.float16
    if "nc" not in _nc_cache:
        _nc_cache["nc"] = _build_nc()
    nc = _nc_cache["nc"]

    xt = np.ascontiguousarray(x[0].T).astype(f16)                 # (C, L)
    posT = _positions_T()                                        # (64, 8192)

    in_maps = []
    for c in range(8):
        h, b = c // 2, c % 2
        w0 = 2048 * (1 - b)
        in_maps.append({
            "xt": xt,
            "xq": np.ascontiguousarray(x[0, b * NQ:(b + 1) * NQ].T).astype(f16),
            "wq": Wq[:, h * DK:(h + 1) * DK].astype(f16),
            "wk": Wk[:, h * DK:(h + 1) * DK].astype(f16),
            "wv": Wv[:, h * DV:(h + 1) * DV].astype(f16),
            "wpos": Wpos[:, h * DK:(h + 1) * DK].astype(f16),
            "post": np.ascontiguousarray(posT[:, w0: w0 + PKW]).astype(f16),
            "wout": Wout[h * DV:(h + 1) * DV, :].astype(f16),
            "rcb": np.ascontiguousarray(
                rel_content_bias[0, h, 0][:, None]).astype(np.float32),
            "rpb": np.ascontiguousarray(
                rel_pos_bias[0, h, 0][:, None]).astype(np.float32),
        })

    res = run_bass_kernel_spmd(nc, in_maps, core_ids=list(range(8)))
    globals()["last_results"] = res
    parts = [r["out"] for r in res.results]

    out = np.zeros((L, C), np.float32)
    for b in range(2):
        acc = np.zeros((NQ, C), np.float32)
        for h in range(4):
            acc += parts[h * 2 + b].astype(np.float32)
        out[b * NQ:(b + 1) * NQ] = acc
    out += bout[None, :].astype(np.float32)
    return out.reshape(1, L, C)
